# revision 1
# baseline (speedup 1.0000x reference)
"""Trainium2 8-core tensor-parallel attention kernel (Bass/Tile).

Sharding: heads tensor-parallel across 8 cores (2 heads/core).
wq/wk/wv column-sharded by head, wo row-sharded; x replicated.
Chunked ReduceScatter (bf16) after the output projection; the host
concatenates the per-core row shards into the full output.

Self-contained: hardcodes B=2, S=2048, DIM=2048, NH=16, HD=128.
"""
import math

import numpy as np

B, S_FULL, DIM, NH = 2, 2048, 2048, 16
HD = 128
N_CORES = 8
HPC = NH // N_CORES          # heads per core
OC = HPC * HD                # output channels per core (256)
DT = DIM // 128              # d-tiles (16)
SC_W = 512                   # schunk width (cols of flattened seq)
RS_ROWS = 512                # rows per ReduceScatter chunk

_CACHE = {}


def _build(S):
    """Build the 8-core SPMD Bass graph for sequence length S (B=2 fixed)."""
    import concourse.bass as bass
    import concourse.mybir as mybir
    import concourse.tile as tile
    from concourse import bacc

    fp32 = mybir.dt.float32
    bf16 = mybir.dt.bfloat16
    Exp = mybir.ActivationFunctionType.Exp
    Copy = mybir.ActivationFunctionType.Copy
    AX = mybir.AxisListType.X
    ADD = mybir.AluOpType.add

    FLAT = B * S                 # flattened rows
    NSC = FLAT // SC_W           # schunks in phase 1
    NQT = S // 128               # q-tiles per batch
    NQG = NQT // 4               # q-groups of 4 tiles per batch
    NCH = FLAT // RS_ROWS        # ReduceScatter chunks
    SCALE = 1.0 / math.sqrt(HD)
    rg = [list(range(N_CORES))]

    nc = bacc.Bacc("TRN2", target_bir_lowering=False, debug=False,
                   num_devices=N_CORES)

    # ---- external parameters ----
    xt_d = nc.declare_dram_parameter("xt", [DIM, FLAT], bf16, isOutput=False)
    wqt_d = nc.declare_dram_parameter("wqt", [DIM, OC], bf16, isOutput=False)
    wkt_d = nc.declare_dram_parameter("wkt", [DIM, OC], bf16, isOutput=False)
    wvt_d = nc.declare_dram_parameter("wvt", [DIM, OC], bf16, isOutput=False)
    wot_d = nc.declare_dram_parameter("wot", [OC, DIM], bf16, isOutput=False)
    cos_d = nc.declare_dram_parameter("cos_t", [HD, S], bf16, isOutput=False)
    sin_d = nc.declare_dram_parameter("sin_t", [HD, S], bf16, isOutput=False)
    mdg_d = nc.declare_dram_parameter("mask_diag", [NQT, 128, 128], fp32, isOutput=False)
    idn_d = nc.declare_dram_parameter("ident_bf", [128, 128], bf16, isOutput=False)
    rot_d = nc.declare_dram_parameter("rotp", [128, 128], bf16, isOutput=False)
    one_d = nc.declare_dram_parameter("ones_bf", [128, 1], bf16, isOutput=False)
    onr_d = nc.declare_dram_parameter("ones_row", [1, 128], fp32, isOutput=False)
    out_d = nc.declare_dram_parameter("out", [NCH, RS_ROWS // N_CORES, DIM], bf16,
                                      isOutput=True)

    # ---- internal DRAM ----
    qT_d = [nc.dram_tensor(f"qT_dram{bb}", [HPC, 128, S], bf16) for bb in range(B)]
    kT_d = [nc.dram_tensor(f"kT_dram{bb}", [HPC, 128, S], bf16) for bb in range(B)]
    vN_d = [nc.dram_tensor(f"vN_dram{bb}", [HPC, S, HD], bf16) for bb in range(B)]
    par_d = [nc.dram_tensor(f"partial_dram{c}", [RS_ROWS, DIM], bf16)
             for c in range(NCH)]
    rs_d = [nc.dram_tensor(f"rs_out{c}", [RS_ROWS // N_CORES, DIM], bf16)
            for c in range(NCH)]

    from contextlib import ExitStack
    with tile.TileContext(nc) as tc:
        with ExitStack() as _stk:
            cpool = _stk.enter_context(tc.tile_pool(name="consts", bufs=1))
            wpool = _stk.enter_context(tc.tile_pool(name="wqkv", bufs=1))
            xpool = _stk.enter_context(tc.tile_pool(name="xT", bufs=10))
            spool = _stk.enter_context(tc.tile_pool(name="p1sb", bufs=3))
            tpool = _stk.enter_context(tc.tile_pool(name="p1tmp", bufs=2))
            qkpool = _stk.enter_context(tc.tile_pool(name="qk_sb", bufs=2))
            vpool = _stk.enter_context(tc.tile_pool(name="vbf", bufs=2))
            ptpool = _stk.enter_context(tc.tile_pool(name="probsT", bufs=2))
            opool = _stk.enter_context(tc.tile_pool(name="outT", bufs=2))
            smpool = _stk.enter_context(tc.tile_pool(name="small", bufs=4))
            papool = _stk.enter_context(tc.tile_pool(name="partial", bufs=4))
            qkvps = tc.alloc_tile_pool(name="qkvps", bufs=6, space="PSUM")
            rotps = tc.alloc_tile_pool(name="rotps", bufs=2, space="PSUM")
            wot_sb = cpool.tile([128, HPC, DIM], bf16)
            nc.gpsimd.dma_start(wot_sb[:], wot_d[:].rearrange("(h p) e -> p h e", p=128))
            cos_sb = cpool.tile([HD, S], bf16)
            nc.gpsimd.dma_start(cos_sb[:], cos_d[:])
            sin_sb = cpool.tile([HD, S], bf16)
            nc.gpsimd.dma_start(sin_sb[:], sin_d[:])
            mdg_sb = cpool.tile([128, NQT, 128], fp32)
            nc.gpsimd.dma_start(mdg_sb[:], mdg_d[:].rearrange("t p k -> p t k"))
            idn_sb = cpool.tile([128, 128], bf16)
            nc.gpsimd.dma_start(idn_sb[:], idn_d[:])
            rot_sb = cpool.tile([128, 128], bf16)
            nc.gpsimd.dma_start(rot_sb[:], rot_d[:])
            one_sb = cpool.tile([128, 1], bf16)
            nc.gpsimd.dma_start(one_sb[:], one_d[:])
            onr_sb = cpool.tile([1, 128], fp32)
            nc.gpsimd.dma_start(onr_sb[:], onr_d[:])

            # ================= phase 1: QKV projections (transposed) ======
            w_sb = {}
            for nm in ("q", "k", "v"):
                w_sb[nm] = wpool.tile([128, DT, OC], bf16, tag=f"w{nm}", name=f"w{nm}")
            # first matmul needs only wq[dt] slabs + x tiles: load those first,
            # one slab at a time, in consumption order
            for dt in range(DT):
                nc.sync.dma_start(w_sb["q"][:, dt, :],
                                  wqt_d[dt * 128:(dt + 1) * 128, :])

            for sc in range(NSC):
                s0 = (sc * SC_W) % S  # position offset within batch
                bb, c0 = divmod(sc * SC_W, S)
                xts = []
                for dt in range(DT):
                    xt = xpool.tile([128, SC_W], bf16, tag="xt", name=f"xt{dt}")
                    nc.sync.dma_start(
                        xt[:], xt_d[dt * 128:(dt + 1) * 128,
                                    sc * SC_W:(sc + 1) * SC_W])
                    xts.append(xt)
                if sc == 0:
                    for dt in range(DT):
                        nc.sync.dma_start(w_sb["k"][:, dt, :],
                                          wkt_d[dt * 128:(dt + 1) * 128, :])
                    for dt in range(DT):
                        nc.sync.dma_start(w_sb["v"][:, dt, :],
                                          wvt_d[dt * 128:(dt + 1) * 128, :])
                for h in range(HPC):
                    ps = {}
                    for t in ("q", "k", "v"):
                        ps[t] = qkvps.tile([128, SC_W], fp32, tag="qkv", name=f"ps_{t}")
                    for t in ("q", "k", "v"):
                        for dt in range(DT):
                            nc.tensor.matmul(
                                ps[t][:],
                                w_sb[t][:, dt, h * HD:(h + 1) * HD],
                                xts[dt][:],
                                start=(dt == 0), stop=(dt == DT - 1))
                    # RoPE for q, k
                    for t, dram, scale in (("q", qT_d, SCALE), ("k", kT_d, 1.0)):
                        til = spool.tile([128, SC_W], bf16, tag="til")
                        nc.scalar.activation(til[:], ps[t][:], Copy,
                                             scale=scale)
                        rp = rotps.tile([128, SC_W], fp32, tag="rot")
                        nc.tensor.matmul(rp[:], rot_sb[:], til[:],
                                         start=True, stop=True)
                        t1 = tpool.tile([128, SC_W], bf16, tag="t1")
                        nc.vector.tensor_mul(t1[:], til[:],
                                             cos_sb[:, s0:s0 + SC_W])
                        hat = spool.tile([128, SC_W], bf16, tag="hat")
                        nc.vector.tensor_mul(hat[:], rp[:],
                                             sin_sb[:, s0:s0 + SC_W])
                        nc.vector.tensor_add(hat[:], hat[:], t1[:])
                        nc.sync.dma_start(dram[bb][h, :, c0:c0 + SC_W], hat[:])
                    # V: copy out of PSUM, then PE-transpose to natural [k, hd]
                    vb = spool.tile([128, SC_W], bf16, tag="vb")
                    nc.scalar.copy(vb[:], ps["v"][:])
                    for vt in range(SC_W // 128):
                        vtp = rotps.tile([128, 128], fp32, tag="rot", name="vtp")
                        nc.tensor.matmul(vtp[:], vb[:, vt * 128:(vt + 1) * 128],
                                         idn_sb[:], start=True, stop=True)
                        vnt = spool.tile([128, 128], bf16, tag="vnt", name="vnt")
                        nc.scalar.copy(vnt[:], vtp[:])
                        nc.sync.dma_start(
                            vN_d[bb][h, c0 + vt * 128: c0 + (vt + 1) * 128, :],
                            vnt[:])

            rotps.release()
            qkvps.release()

            # ================= phase 2: attention + O-proj + RS ===========
            with ExitStack() as _stk2:
                scps = _stk2.enter_context(tc.tile_pool(name="scps", bufs=2, space="PSUM"))
                bps = _stk2.enter_context(tc.tile_pool(name="bps", bufs=1, space="PSUM"))
                sups = _stk2.enter_context(tc.tile_pool(name="sups", bufs=1, space="PSUM"))
                ops = _stk2.enter_context(tc.tile_pool(name="ops", bufs=2, space="PSUM"))
                pps = _stk2.enter_context(tc.tile_pool(name="pps", bufs=2, space="PSUM"))
                def attn_group2(qg, kmax, qT, kT, vbf, oT):
                    pT = {h: ptpool.tile([128, NQT, 512], bf16, tag="pT",
                                         name=f"pT{h}") for h in range(HPC)}
                    po = {h: ops.tile([128, 512], fp32, tag="po",
                                      name=f"po{h}") for h in range(HPC)}
                    for kt in range(kmax + 1):
                        qlo = max(0, kt - qg * 4) * 128
                        n = 512 - qlo
                        for h in range(HPC):
                            sp = scps.tile([128, 512], fp32, tag="sc", name="sp")
                            nc.tensor.matmul(
                                sp[:, :n],
                                kT[h][:, kt * 128:(kt + 1) * 128],
                                qT[h][:, qg * 512 + qlo: (qg + 1) * 512],
                                start=True, stop=True)
                            if kt >= qg * 4:  # diag tile at local cols 0:128
                                nc.vector.tensor_add(
                                    sp[:, 0:128], sp[:, 0:128], mdg_sb[:, kt, :])
                            nc.scalar.activation(
                                pT[h][:, kt, qlo:512], sp[:, :n], Exp)
                            if kt >= 1:
                                klast = kt - 1
                                ql2 = max(0, klast - qg * 4) * 128
                                nc.tensor.matmul(
                                    po[h][:, ql2:512], vbf[h][:, klast, :],
                                    pT[h][:, klast, ql2:512],
                                    start=(klast == 0), stop=False)
                    for h in range(HPC):
                        nc.tensor.matmul(
                            po[h][:, 384:512], vbf[h][:, kmax, :],
                            pT[h][:, kmax, 384:512], start=False, stop=True)
                        sums_ps = sups.tile([1, 512], fp32, tag="sps", name="sums_ps")
                        for kt in range(kmax + 1):
                            qlo = max(0, kt - qg * 4) * 128
                            nc.tensor.matmul(
                                sums_ps[:, qlo:512], one_sb[:],
                                pT[h][:, kt, qlo:512],
                                start=(kt == 0), stop=(kt == kmax))
                        srow = smpool.tile([1, 512], fp32, tag="srow", name="srow")
                        nc.scalar.copy(srow[:], sums_ps[:])
                        sbc_ps = bps.tile([128, 512], fp32, tag="sbc", name="sbc_ps")
                        nc.tensor.matmul(sbc_ps[:], onr_sb[:], srow[:],
                                         start=True, stop=True)
                        rbc = smpool.tile([128, 512], fp32, tag="rbc", name="rbc")
                        nc.vector.reciprocal_approx_fast(rbc[:], sbc_ps[:])
                        nc.vector.tensor_mul(
                            oT[h][:, qg * 512:(qg + 1) * 512], po[h][:], rbc[:])

                qTa, kTa, vbfa = {}, {}, {}
                for bb2 in range(B):
                    for h in range(HPC):
                        qTa[(bb2, h)] = qkpool.tile([128, S], bf16, tag=f"qT{h}", name=f"qT{bb2}{h}")
                        nc.sync.dma_start(qTa[(bb2, h)][:], qT_d[bb2][h])
                        kTa[(bb2, h)] = qkpool.tile([128, S], bf16, tag=f"kT{h}", name=f"kT{bb2}{h}")
                        nc.sync.dma_start(kTa[(bb2, h)][:], kT_d[bb2][h])
                        vbfa[(bb2, h)] = vpool.tile([128, NQT, HD], bf16, tag=f"v{h}", name=f"v{bb2}{h}")
                        nc.sync.dma_start(
                            vbfa[(bb2, h)][:],
                            vN_d[bb2][h].rearrange("(t p) d -> p t d", p=128))

                oTa = {}
                for bb2 in range(B):
                    for h in range(HPC):
                        oTa[(bb2, h)] = opool.tile([128, S], bf16, tag=f"oT{h}",
                                                   name=f"oT{bb2}{h}")

                for qg in range(NQG):
                    kmax = qg * 4 + 3
                    for b in range(B):
                        qT = {h: qTa[(b, h)] for h in range(HPC)}
                        kT = {h: kTa[(b, h)] for h in range(HPC)}
                        vbf = {h: vbfa[(b, h)] for h in range(HPC)}
                        oT = {h: oTa[(b, h)] for h in range(HPC)}
                        attn_group2(qg, kmax, qT, kT, vbf, oT)

                        # ---- O-projection for this q-group + ReduceScatter ----
                        for st in range(qg * 4, qg * 4 + 4):
                            pp = [pps.tile([128, 512], fp32, tag="pp", name=f"pp{e}") for e in range(4)]
                            for h in range(HPC):
                                for ec in range(4):
                                    nc.tensor.matmul(
                                        pp[ec][:],
                                        oT[h][:, st * 128:(st + 1) * 128],
                                        wot_sb[:, h, ec * 512:(ec + 1) * 512],
                                        start=(h == 0), stop=(h == HPC - 1))
                            par = papool.tile([128, DIM], bf16, tag="par")
                            chx, r0 = divmod(b * S + st * 128, RS_ROWS)
                            for ec in range(4):
                                if ec % 2 == 0:
                                    nc.scalar.copy(par[:, ec * 512:(ec + 1) * 512], pp[ec][:])
                                else:
                                    nc.vector.tensor_copy(par[:, ec * 512:(ec + 1) * 512], pp[ec][:])
                            nc.sync.dma_start(par_d[chx][r0:r0 + 128, :], par[:])
                            if r0 + 128 == RS_ROWS:
                                nc.gpsimd.collective_compute(
                                    "ReduceScatter", ADD, replica_groups=rg,
                                    ins=[par_d[chx][:]],
                                    outs=[rs_d[chx][:]])
                                nc.gpsimd.dma_start(out_d[chx], rs_d[chx][:])

    nc.compile()
    return nc


def _get_nc(S):
    if S not in _CACHE:
        _CACHE[S] = _build(S)
    return _CACHE[S]


def make_inputs(x, freqs_cis, mask, wq, wk, wv, wo):
    """Host-side sharding / layout prep. Returns in_maps for 8 cores."""
    S = x.shape[1]
    flat_xt = np.ascontiguousarray(np.asarray(x, np.float32).reshape(B * S, DIM).T)
    cos = np.asarray(freqs_cis[..., 0], np.float32)   # [S, HD/2]
    sin = np.asarray(freqs_cis[..., 1], np.float32)
    cos_t = np.ascontiguousarray(np.repeat(cos.T, 2, axis=0))  # [HD, S]
    sin_t = np.ascontiguousarray(np.repeat(sin.T, 2, axis=0))
    m = np.asarray(mask, np.float32)[0, 0]
    nqt = S // 128
    mask_diag = np.ascontiguousarray(
        np.stack([m[i * 128:(i + 1) * 128, i * 128:(i + 1) * 128].T
                  for i in range(nqt)]))
    import ml_dtypes
    bf = ml_dtypes.bfloat16
    flat_xt = flat_xt.astype(bf)
    cos_t = cos_t.astype(bf)
    sin_t = sin_t.astype(bf)
    ident_bf = np.eye(128, dtype=bf)
    P = np.zeros((128, 128), np.float32)
    for j in range(64):
        P[2 * j, 2 * j + 1] = -1.0
        P[2 * j + 1, 2 * j] = 1.0
    rotp = np.ascontiguousarray(P.T)

    in_maps = []
    for c in range(N_CORES):
        r = slice(c * OC, (c + 1) * OC)
        in_maps.append({
            "xt": flat_xt,
            "wqt": np.ascontiguousarray(np.asarray(wq, np.float32)[r, :].T).astype(bf),
            "wkt": np.ascontiguousarray(np.asarray(wk, np.float32)[r, :].T).astype(bf),
            "wvt": np.ascontiguousarray(np.asarray(wv, np.float32)[r, :].T).astype(bf),
            "wot": np.ascontiguousarray(np.asarray(wo, np.float32)[:, r].T).astype(bf),
            "cos_t": cos_t,
            "sin_t": sin_t,
            "mask_diag": mask_diag,
            "ident_bf": ident_bf,
            "rotp": rotp.astype(bf),
            "ones_bf": np.ones((128, 1), dtype=bf),
            "ones_row": np.ones((1, 128), dtype=np.float32),
        })
    return in_maps


def assemble(results, S):
    """Concatenate per-core ReduceScatter shards into the full output."""
    nch = B * S // RS_ROWS
    per = RS_ROWS // N_CORES
    full = np.empty((nch, N_CORES, per, DIM), np.float32)
    for c in range(N_CORES):
        full[:, c] = np.asarray(results[c]["out"], np.float32).reshape(nch, per, DIM)
    return full.reshape(B, S, DIM)


def kernel(x, start_pos, freqs_cis, mask, wq, wk, wv, wo):
    from concourse.bass_utils import run_bass_kernel_spmd
    S = x.shape[1]
    nc = _get_nc(S)
    in_maps = make_inputs(x, freqs_cis, mask, wq, wk, wv, wo)
    res = run_bass_kernel_spmd(nc, in_maps, core_ids=list(range(N_CORES)))
    return assemble(res.results, S)



# revision 4
# speedup vs baseline: 1.2047x; 1.2047x over previous
"""Trainium2 8-core tensor-parallel attention kernel (Bass/Tile), v2.

Sharding: heads tensor-parallel across 8 cores (2 heads/core) for
QKV + attention; output projection is column-sharded (each core owns
256 output channels) fed by per-chunk AllGathers of the pre-projection
attention outputs (2MB total exchanged vs 16.8MB for post-wo
ReduceScatter).

Structure (single fused phase, interleaved for PE occupancy):
  for sc in 0..7:            # 512 flat seq rows each; b = sc//4
    QKV projections for this schunk (Q/K in [hd,seq] + RoPE, V in
    natural [seq,hd] layout directly), all SBUF-resident
    attention q-group (qg = sc%4) for batch b; softmax sums via
    gpsimd accumulation + one ones128 broadcast-sum matmul
    AllGather of this chunk's attention outputs (8 cores x 256 rows)
    O-projection of chunk sc-2 (delayed so the AllGather is off the
    PE critical path)

Self-contained: hardcodes B=2, S=2048, DIM=2048, NH=16, HD=128.
"""
import math

import numpy as np

B, S_FULL, DIM, NH = 2, 2048, 2048, 16
HD = 128
N_CORES = 8
HPC = NH // N_CORES          # heads per core (2)
OC = HPC * HD                # q/k/v channels per core (256)
OCD = DIM // N_CORES         # output channels per core (256)
DT = DIM // 128              # dim tiles (16)
SC_W = 512                   # schunk width (cols of flattened seq)

_CACHE = {}


def _build(S):
    """Build the 8-core SPMD Bass graph for sequence length S (B=2 fixed)."""
    import concourse.bass as bass
    import concourse.mybir as mybir
    import concourse.tile as tile
    from concourse import bacc

    fp32 = mybir.dt.float32
    bf16 = mybir.dt.bfloat16
    Exp = mybir.ActivationFunctionType.Exp
    Copy = mybir.ActivationFunctionType.Copy
    BYPASS = mybir.AluOpType.bypass

    FLAT = B * S                 # flattened rows (4096)
    NSC = FLAT // SC_W           # schunks / chunks (8)
    NQT = S // 128               # k-tiles per batch (16)
    SCALE = 1.0 / math.sqrt(HD)
    rg = [list(range(N_CORES))]

    nc = bacc.Bacc("TRN2", target_bir_lowering=False, debug=False,
                   num_devices=N_CORES)

    # ---- external parameters ----
    xt_d = nc.declare_dram_parameter("xt", [DIM, FLAT], bf16, isOutput=False)
    wqt_d = nc.declare_dram_parameter("wqt", [DIM, OC], bf16, isOutput=False)
    wkt_d = nc.declare_dram_parameter("wkt", [DIM, OC], bf16, isOutput=False)
    wvt_d = nc.declare_dram_parameter("wvt", [DIM, OC], bf16, isOutput=False)
    wot_d = nc.declare_dram_parameter("wotc", [DIM, OCD], bf16, isOutput=False)
    cos_d = nc.declare_dram_parameter("cos_t", [HD, S], bf16, isOutput=False)
    sin_d = nc.declare_dram_parameter("sin_t", [HD, S], bf16, isOutput=False)
    mdg_d = nc.declare_dram_parameter("mask_diag", [NQT, 128, 128], bf16, isOutput=False)
    rot_d = nc.declare_dram_parameter("rotp", [128, 128], bf16, isOutput=False)
    on2_d = nc.declare_dram_parameter("ones128", [128, 128], bf16, isOutput=False)
    out_d = nc.declare_dram_parameter("outT", [OCD, FLAT], bf16, isOutput=True)

    # ---- internal DRAM (collective staging) ----
    ag_in_d = [nc.dram_tensor(f"ag_in{p}", [OC, SC_W], bf16) for p in range(NSC)]
    ag_out_d = [nc.dram_tensor(f"ag_out{p}", [N_CORES * OC, SC_W], bf16,
                               addr_space="Shared") for p in range(NSC)]

    from contextlib import ExitStack
    with tile.TileContext(nc) as tc:
        with ExitStack() as _stk:
            cpool = _stk.enter_context(tc.tile_pool(name="consts", bufs=1))
            wpool = _stk.enter_context(tc.tile_pool(name="wqkv", bufs=1))
            xpool = _stk.enter_context(tc.tile_pool(name="xT", bufs=32))
            qkpool = _stk.enter_context(tc.tile_pool(name="qk_sb", bufs=1))
            vpool = _stk.enter_context(tc.tile_pool(name="vbf", bufs=1))
            spool = _stk.enter_context(tc.tile_pool(name="p1tmp", bufs=3))
            tpool = _stk.enter_context(tc.tile_pool(name="t1tmp", bufs=2))
            ptpool = _stk.enter_context(tc.tile_pool(name="probsT", bufs=4))
            acpool = _stk.enter_context(tc.tile_pool(name="accs", bufs=2))
            smpool = _stk.enter_context(tc.tile_pool(name="small", bufs=2))
            aglpool = _stk.enter_context(tc.tile_pool(name="agl", bufs=20))
            obpool = _stk.enter_context(tc.tile_pool(name="outsb", bufs=4))
            qkvps = _stk.enter_context(tc.tile_pool(name="qkvps", bufs=2, space="PSUM"))
            rotps = _stk.enter_context(tc.tile_pool(name="rotps", bufs=1, space="PSUM"))
            scps = _stk.enter_context(tc.tile_pool(name="scps", bufs=2, space="PSUM"))
            pops = _stk.enter_context(tc.tile_pool(name="pops", bufs=2, space="PSUM"))
            opps = _stk.enter_context(tc.tile_pool(name="opps", bufs=1, space="PSUM"))

            # ---- weights first (critical path: first matmul needs wq) ----
            w_sb = {}
            for nm in ("q", "k", "v"):
                w_sb[nm] = wpool.tile([128, DT, OC], bf16, tag=f"w{nm}", name=f"w{nm}")
            for dt in range(DT):
                nc.sync.dma_start(w_sb["q"][:, dt, :],
                                  wqt_d[dt * 128:(dt + 1) * 128, :])

            # persistent SBUF tensors
            qTa, kTa, vbfa = {}, {}, {}
            for bb in range(B):
                for h in range(HPC):
                    qTa[(bb, h)] = qkpool.tile([128, S], bf16, tag=f"qT{bb}{h}",
                                               name=f"qT{bb}{h}")
                    kTa[(bb, h)] = qkpool.tile([128, S], bf16, tag=f"kT{bb}{h}",
                                               name=f"kT{bb}{h}")
                    vbfa[(bb, h)] = vpool.tile([128, NQT, HD], bf16,
                                               tag=f"v{bb}{h}", name=f"v{bb}{h}")

            cos_sb = cpool.tile([HD, S], bf16)
            sin_sb = cpool.tile([HD, S], bf16)
            mdg_sb = cpool.tile([128, NQT, 128], bf16)
            rot_sb = cpool.tile([128, 128], bf16)
            on2_sb = cpool.tile([128, 128], bf16)
            wot_sb = cpool.tile([128, DT, OCD], bf16)

            def load_consts():
                for dt in range(DT):
                    nc.sync.dma_start(w_sb["k"][:, dt, :],
                                      wkt_d[dt * 128:(dt + 1) * 128, :])
                for dt in range(DT):
                    nc.sync.dma_start(w_sb["v"][:, dt, :],
                                      wvt_d[dt * 128:(dt + 1) * 128, :])
                nc.gpsimd.dma_start(cos_sb[:], cos_d[:])
                nc.gpsimd.dma_start(sin_sb[:], sin_d[:])
                nc.gpsimd.dma_start(mdg_sb[:], mdg_d[:].rearrange("t p k -> p t k"))
                nc.gpsimd.dma_start(rot_sb[:], rot_d[:])
                nc.gpsimd.dma_start(on2_sb[:], on2_d[:])
                for dt in range(DT):
                    nc.sync.dma_start(wot_sb[:, dt, :],
                                      wot_d[dt * 128:(dt + 1) * 128, :])

            # ============ per-chunk attention ============
            def attn_chunk(qg, b):
                kmax = qg * 4 + 3
                p = b * 4 + qg
                for h in range(HPC):
                    po_ps = pops.tile([128, SC_W], fp32, tag="po", name=f"po{h}")
                    acc = acpool.tile([128, SC_W], fp32, tag="acc", name=f"acc{h}")
                    pts = {}
                    for kt in range(kmax + 1):
                        qlo = max(0, kt - qg * 4) * 128
                        n = SC_W - qlo
                        sp = scps.tile([128, SC_W], fp32, tag="sc", name="sp")
                        nc.tensor.matmul(
                            sp[:, :n],
                            kTa[(b, h)][:, kt * 128:(kt + 1) * 128],
                            qTa[(b, h)][:, qg * SC_W + qlo:(qg + 1) * SC_W],
                            start=True, stop=True)
                        if kt >= qg * 4:
                            nc.vector.tensor_add(
                                sp[:, 0:128], sp[:, 0:128], mdg_sb[:, kt, :])
                        pt = ptpool.tile([128, SC_W], bf16, tag=f"pT{h}",
                                         name=f"pT{h}")
                        pts[kt] = (pt, qlo)
                        nc.scalar.activation(pt[:, qlo:SC_W], sp[:, :n], Exp)
                        if kt == 0:
                            nc.gpsimd.tensor_copy(acc[:], pt[:])
                        else:
                            nc.gpsimd.tensor_add(acc[:, qlo:SC_W],
                                                 acc[:, qlo:SC_W],
                                                 pt[:, qlo:SC_W])
                        if kt >= 1:
                            kl = kt - 1
                            ptl, ql2 = pts[kl]
                            nc.tensor.matmul(
                                po_ps[:, ql2:SC_W], vbfa[(b, h)][:, kl, :],
                                ptl[:, ql2:SC_W],
                                start=(kl == 0), stop=False)
                    ptl, ql2 = pts[kmax]
                    nc.tensor.matmul(
                        po_ps[:, ql2:SC_W], vbfa[(b, h)][:, kmax, :],
                        ptl[:, ql2:SC_W], start=(kmax == 0), stop=True)
                    # broadcast softmax denominators: ones128 @ acc_bf
                    accb = tpool.tile([128, SC_W], bf16, tag="accbf", name="accb")
                    nc.scalar.copy(accb[:], acc[:])
                    sb_ps = rotps.tile([128, SC_W], fp32, tag="rot", name="sb_ps")
                    nc.tensor.matmul(sb_ps[:], on2_sb[:], accb[:],
                                     start=True, stop=True)
                    rbc = smpool.tile([128, SC_W], fp32, tag="rbc", name="rbc")
                    nc.vector.reciprocal_approx_fast(rbc[:], sb_ps[:])
                    ob = smpool.tile([128, SC_W], bf16, tag="obuf", name="ob")
                    nc.vector.tensor_mul(ob[:], po_ps[:], rbc[:])
                    nc.sync.dma_start(ag_in_d[p][h * 128:(h + 1) * 128, :], ob[:])
                nc.gpsimd.collective_compute(
                    "AllGather", BYPASS, replica_groups=rg,
                    ins=[ag_in_d[p][:]], outs=[ag_out_d[p][:]])

            # ============ per-chunk O-projection (column-sharded) ============
            def oproj_chunk(p):
                slabs = []
                for e in range(DT):
                    agl = aglpool.tile([128, SC_W], bf16, tag="agl",
                                       name=f"agl{e}")
                    nc.sync.dma_start(agl[:],
                                      ag_out_d[p][e * 128:(e + 1) * 128, :])
                    slabs.append(agl)
                for dh in range(2):
                    op_ps = opps.tile([128, SC_W], fp32, tag="op", name="op_ps")
                    for e in range(DT):
                        nc.tensor.matmul(
                            op_ps[:],
                            wot_sb[:, e, dh * 128:(dh + 1) * 128],
                            slabs[e][:],
                            start=(e == 0), stop=(e == DT - 1))
                    obt = obpool.tile([128, SC_W], bf16, tag="ob", name="obt")
                    if dh == 0:
                        nc.scalar.copy(obt[:], op_ps[:])
                    else:
                        nc.vector.tensor_copy(obt[:], op_ps[:])
                    nc.sync.dma_start(
                        out_d[dh * 128:(dh + 1) * 128,
                              p * SC_W:(p + 1) * SC_W], obt[:])

            # ============ main fused loop ============
            for sc in range(NSC):
                b, qg = divmod(sc, 4)
                c0 = qg * SC_W               # column offset within batch
                xts = []
                for dt in range(DT):
                    xt = xpool.tile([128, SC_W], bf16, tag="xt", name=f"xt{dt}")
                    nc.sync.dma_start(
                        xt[:], xt_d[dt * 128:(dt + 1) * 128,
                                    sc * SC_W:(sc + 1) * SC_W])
                    xts.append(xt)
                if sc == 0:
                    load_consts()

                rope_pend = []   # deferred PE ops to avoid stalls

                def emit_rope_mm(t, h, til):
                    rp = rotps.tile([128, SC_W], fp32, tag="rot", name="rp")
                    nc.tensor.matmul(rp[:], rot_sb[:], til[:],
                                     start=True, stop=True)
                    dst = (qTa if t == "q" else kTa)[(b, h)]
                    t1 = tpool.tile([128, SC_W], bf16, tag="t1", name="t1")
                    nc.vector.tensor_mul(t1[:], til[:], cos_sb[:, c0:c0 + SC_W])
                    hat = spool.tile([128, SC_W], bf16, tag="hat", name="hat")
                    nc.vector.tensor_mul(hat[:], rp[:], sin_sb[:, c0:c0 + SC_W])
                    nc.vector.tensor_add(dst[:, c0:c0 + SC_W], hat[:], t1[:])

                for h in range(HPC):
                    tils = {}
                    for t in ("q", "k"):
                        ps = qkvps.tile([128, SC_W], fp32, tag="qkv",
                                        name=f"ps_{t}")
                        for dt in range(DT):
                            nc.tensor.matmul(
                                ps[:],
                                w_sb[t][:, dt, h * HD:(h + 1) * HD],
                                xts[dt][:],
                                start=(dt == 0), stop=(dt == DT - 1))
                        til = spool.tile([128, SC_W], bf16, tag="til",
                                         name=f"til_{t}")
                        if t == "q":
                            nc.scalar.activation(til[:], ps[:], Copy,
                                                 scale=SCALE)
                        else:
                            nc.scalar.copy(til[:], ps[:])
                        tils[t] = til
                        if t == "k":
                            # q's rope matmul: til_q ready by now (k chain
                            # gave the scalar engine 4us of cover)
                            emit_rope_mm("q", h, tils["q"])
                    if h == 1:
                        # v of h==0..1 is shared (both heads in one pass);
                        # emitted once below
                        emit_rope_mm("k", h, tils["k"])
                    else:
                        # natural-layout V: stationary = x tile, both heads
                        # at once; no scalar input -> no PE stall risk
                        for vt in range(4):
                            psv = qkvps.tile([128, OC], fp32, tag="qkv",
                                             name="ps_v")
                            for dt in range(DT):
                                nc.tensor.matmul(
                                    psv[:],
                                    xts[dt][:, vt * 128:(vt + 1) * 128],
                                    w_sb["v"][:, dt, :],
                                    start=(dt == 0), stop=(dt == DT - 1))
                            for hh in range(HPC):
                                if hh == 0:
                                    nc.scalar.copy(
                                        vbfa[(b, hh)][:, qg * 4 + vt, :],
                                        psv[:, hh * HD:(hh + 1) * HD])
                                else:
                                    nc.vector.tensor_copy(
                                        vbfa[(b, hh)][:, qg * 4 + vt, :],
                                        psv[:, hh * HD:(hh + 1) * HD])
                        emit_rope_mm("k", h, tils["k"])

                attn_chunk(qg, b)
                if sc >= 2:
                    oproj_chunk(sc - 2)
            oproj_chunk(NSC - 2)
            oproj_chunk(NSC - 1)

    nc.compile()
    return nc


def _get_nc(S):
    if S not in _CACHE:
        _CACHE[S] = _build(S)
    return _CACHE[S]


def make_inputs(x, freqs_cis, mask, wq, wk, wv, wo):
    """Host-side sharding / layout prep. Returns in_maps for 8 cores."""
    S = x.shape[1]
    flat_xt = np.ascontiguousarray(np.asarray(x, np.float32).reshape(B * S, DIM).T)
    cos = np.asarray(freqs_cis[..., 0], np.float32)   # [S, HD/2]
    sin = np.asarray(freqs_cis[..., 1], np.float32)
    cos_t = np.ascontiguousarray(np.repeat(cos.T, 2, axis=0))  # [HD, S]
    sin_t = np.ascontiguousarray(np.repeat(sin.T, 2, axis=0))
    m = np.asarray(mask, np.float32)[0, 0]
    nqt = S // 128
    mask_diag = np.ascontiguousarray(
        np.stack([m[i * 128:(i + 1) * 128, i * 128:(i + 1) * 128].T
                  for i in range(nqt)]))
    import ml_dtypes
    bf = ml_dtypes.bfloat16
    flat_xt = flat_xt.astype(bf)
    cos_t = cos_t.astype(bf)
    sin_t = sin_t.astype(bf)
    P = np.zeros((128, 128), np.float32)
    for j in range(64):
        P[2 * j, 2 * j + 1] = -1.0
        P[2 * j + 1, 2 * j] = 1.0
    rotp = np.ascontiguousarray(P.T)

    wq = np.asarray(wq, np.float32)
    wk = np.asarray(wk, np.float32)
    wv = np.asarray(wv, np.float32)
    wo = np.asarray(wo, np.float32)
    in_maps = []
    for c in range(N_CORES):
        r = slice(c * OC, (c + 1) * OC)
        rd = slice(c * OCD, (c + 1) * OCD)
        in_maps.append({
            "xt": flat_xt,
            "wqt": np.ascontiguousarray(wq[r, :].T).astype(bf),
            "wkt": np.ascontiguousarray(wk[r, :].T).astype(bf),
            "wvt": np.ascontiguousarray(wv[r, :].T).astype(bf),
            "wotc": np.ascontiguousarray(wo[rd, :].T).astype(bf),
            "cos_t": cos_t,
            "sin_t": sin_t,
            "mask_diag": mask_diag.astype(bf),
            "rotp": rotp.astype(bf),
            "ones128": np.ones((128, 128), dtype=bf),
        })
    return in_maps


def assemble(results, S):
    """Column-concat per-core output shards into the full output."""
    full = np.empty((B * S, DIM), np.float32)
    for c in range(N_CORES):
        full[:, c * OCD:(c + 1) * OCD] = \
            np.asarray(results[c]["outT"], np.float32).T
    return full.reshape(B, S, DIM)


def kernel(x, start_pos, freqs_cis, mask, wq, wk, wv, wo):
    from concourse.bass_utils import run_bass_kernel_spmd
    S = x.shape[1]
    nc = _get_nc(S)
    in_maps = make_inputs(x, freqs_cis, mask, wq, wk, wv, wo)
    res = run_bass_kernel_spmd(nc, in_maps, core_ids=list(range(N_CORES)))
    return assemble(res.results, S)


# revision 5
# speedup vs baseline: 1.3046x; 1.0829x over previous
"""Trainium2 8-core tensor-parallel attention kernel (Bass/Tile), v3.

Sharding: heads tensor-parallel across 8 cores (2 heads/core) for
QKV + attention; output projection is column-sharded (each core owns
256 output channels) fed by per-chunk AllGathers of the pre-projection
attention outputs (2MB total exchanged vs 16.8MB for post-wo
ReduceScatter).

Single fused loop, interleaved to keep the PE p-state ramped:
  for sc in 0..7:                  # 512 flat seq rows; b = sc//4
    QKV q-chain for h=0
    normalize + AllGather of chunk sc-1   (softmax sums ready by now)
    rest of QKV (Q/K in [hd,seq] + RoPE, V in natural [seq,hd])
    attention q-group (scores/exp/AV; sums via in-place bf16 tree
    reduction on the contiguous probs buffer)
    O-projection of chunk sc-3            (AllGather long since done)

Self-contained: hardcodes B=2, S=2048, DIM=2048, NH=16, HD=128.
"""
import math

import numpy as np

B, S_FULL, DIM, NH = 2, 2048, 2048, 16
HD = 128
N_CORES = 8
HPC = NH // N_CORES          # heads per core (2)
OC = HPC * HD                # q/k/v channels per core (256)
OCD = DIM // N_CORES         # output channels per core (256)
DT = DIM // 128              # dim tiles (16)
SC_W = 512                   # schunk width (cols of flattened seq)

_CACHE = {}


def _build(S):
    """Build the 8-core SPMD Bass graph for sequence length S (B=2 fixed)."""
    import concourse.bass as bass
    import concourse.mybir as mybir
    import concourse.tile as tile
    from concourse import bacc

    fp32 = mybir.dt.float32
    bf16 = mybir.dt.bfloat16
    Exp = mybir.ActivationFunctionType.Exp
    Copy = mybir.ActivationFunctionType.Copy
    BYPASS = mybir.AluOpType.bypass

    FLAT = B * S                 # flattened rows (4096)
    NSC = FLAT // SC_W           # schunks / chunks (8)
    NQT = S // 128               # k-tiles per batch (16)
    SCALE = 1.0 / math.sqrt(HD)
    rg = [list(range(N_CORES))]

    nc = bacc.Bacc("TRN2", target_bir_lowering=False, debug=False,
                   num_devices=N_CORES)

    # ---- external parameters ----
    xt_d = nc.declare_dram_parameter("xt", [DIM, FLAT], bf16, isOutput=False)
    wqt_d = nc.declare_dram_parameter("wqt", [DIM, OC], bf16, isOutput=False)
    wkt_d = nc.declare_dram_parameter("wkt", [DIM, OC], bf16, isOutput=False)
    wvt_d = nc.declare_dram_parameter("wvt", [DIM, OC], bf16, isOutput=False)
    wot_d = nc.declare_dram_parameter("wotc", [DIM, OCD], bf16, isOutput=False)
    cos_d = nc.declare_dram_parameter("cos_t", [HD, S], bf16, isOutput=False)
    sin_d = nc.declare_dram_parameter("sin_t", [HD, S], bf16, isOutput=False)
    mdg_d = nc.declare_dram_parameter("mask_diag", [NQT, 128, 128], bf16, isOutput=False)
    rot_d = nc.declare_dram_parameter("rotp", [128, 128], bf16, isOutput=False)
    on2_d = nc.declare_dram_parameter("ones128", [128, 128], bf16, isOutput=False)
    out_d = nc.declare_dram_parameter("outT", [OCD, FLAT], bf16, isOutput=True)

    # ---- internal DRAM (collective staging) ----
    ag_in_d = [nc.dram_tensor(f"ag_in{p}", [OC, SC_W], bf16) for p in range(NSC)]
    ag_out_d = [nc.dram_tensor(f"ag_out{p}", [N_CORES * OC, SC_W], bf16,
                               addr_space="Shared") for p in range(NSC)]

    from contextlib import ExitStack
    with tile.TileContext(nc) as tc:
        with ExitStack() as _stk:
            cpool = _stk.enter_context(tc.tile_pool(name="consts", bufs=1))
            wpool = _stk.enter_context(tc.tile_pool(name="wqkv", bufs=1))
            xpool = _stk.enter_context(tc.tile_pool(name="xT", bufs=24))
            qkpool = _stk.enter_context(tc.tile_pool(name="qk_sb", bufs=1))
            vpool = _stk.enter_context(tc.tile_pool(name="vbf", bufs=1))
            spool = _stk.enter_context(tc.tile_pool(name="p1tmp", bufs=3))
            tpool = _stk.enter_context(tc.tile_pool(name="t1tmp", bufs=2))
            ptpool = _stk.enter_context(tc.tile_pool(name="probsT", bufs=1))
            smpool = _stk.enter_context(tc.tile_pool(name="small", bufs=2))
            aglpool = _stk.enter_context(tc.tile_pool(name="agl", bufs=20))
            obpool = _stk.enter_context(tc.tile_pool(name="outsb", bufs=4))
            qkvps = _stk.enter_context(tc.tile_pool(name="qkvps", bufs=2, space="PSUM"))
            rotps = _stk.enter_context(tc.tile_pool(name="rotps", bufs=1, space="PSUM"))
            scps = _stk.enter_context(tc.tile_pool(name="scps", bufs=2, space="PSUM"))
            pops = _stk.enter_context(tc.tile_pool(name="pops", bufs=2, space="PSUM"))
            opps = _stk.enter_context(tc.tile_pool(name="opps", bufs=1, space="PSUM"))

            # ---- weights + first x tiles interleaved (critical path) ----
            w_sb = {}
            for nm in ("q", "k", "v"):
                w_sb[nm] = wpool.tile([128, DT, OC], bf16, tag=f"w{nm}", name=f"w{nm}")
            xts0 = []
            for dt in range(DT):
                nc.sync.dma_start(w_sb["q"][:, dt, :],
                                  wqt_d[dt * 128:(dt + 1) * 128, :])
                xt = xpool.tile([128, SC_W], bf16, tag="xt", name=f"xt{dt}")
                nc.sync.dma_start(xt[:], xt_d[dt * 128:(dt + 1) * 128, 0:SC_W])
                xts0.append(xt)

            # persistent SBUF tensors
            qTa, kTa, vbfa = {}, {}, {}
            for bb in range(B):
                for h in range(HPC):
                    qTa[(bb, h)] = qkpool.tile([128, S], bf16, tag=f"qT{bb}{h}",
                                               name=f"qT{bb}{h}")
                    kTa[(bb, h)] = qkpool.tile([128, S], bf16, tag=f"kT{bb}{h}",
                                               name=f"kT{bb}{h}")
                    vbfa[(bb, h)] = vpool.tile([128, NQT, HD], bf16,
                                               tag=f"v{bb}{h}", name=f"v{bb}{h}")

            cos_sb = cpool.tile([HD, S], bf16)
            sin_sb = cpool.tile([HD, S], bf16)
            mdg_sb = cpool.tile([128, NQT, 128], bf16)
            rot_sb = cpool.tile([128, 128], bf16)
            on2_sb = cpool.tile([128, 128], bf16)
            wot_sb = cpool.tile([128, DT, OCD], bf16)

            def load_consts():
                for dt in range(DT):
                    nc.sync.dma_start(w_sb["k"][:, dt, :],
                                      wkt_d[dt * 128:(dt + 1) * 128, :])
                for dt in range(DT):
                    nc.sync.dma_start(w_sb["v"][:, dt, :],
                                      wvt_d[dt * 128:(dt + 1) * 128, :])
                nc.gpsimd.dma_start(cos_sb[:], cos_d[:])
                nc.gpsimd.dma_start(sin_sb[:], sin_d[:])
                nc.gpsimd.dma_start(mdg_sb[:], mdg_d[:].rearrange("t p k -> p t k"))
                nc.gpsimd.dma_start(rot_sb[:], rot_d[:])
                nc.gpsimd.dma_start(on2_sb[:], on2_d[:])
                for dt in range(DT):
                    nc.sync.dma_start(wot_sb[:, dt, :],
                                      wot_d[dt * 128:(dt + 1) * 128, :])

            # state carried between loop iterations for deferred normalize
            pending = {}   # chunk p -> (qg, b, {h: (pt_buf, po_ps)})

            # ============ per-chunk attention (h sections only) ============
            def attn_chunk(qg, b):
                kmax = qg * 4 + 3
                K = kmax + 1
                p = b * 4 + qg
                hstate = {}
                for h in range(HPC):
                    po_ps = pops.tile([128, SC_W], fp32, tag="po", name=f"po{h}")
                    ptb = ptpool.tile([128, NQT, SC_W], bf16, tag=f"pTb{h}",
                                      name=f"pTb{h}")
                    # zero the above-diagonal cols of the partial tiles so the
                    # tree reduction sees exact zeros there
                    for kt in range(qg * 4 + 1, kmax + 1):
                        qlo = (kt - qg * 4) * 128
                        nc.gpsimd.memset(ptb[:, kt, 0:qlo], 0)
                    for kt in range(K):
                        qlo = max(0, kt - qg * 4) * 128
                        n = SC_W - qlo
                        sp = scps.tile([128, SC_W], fp32, tag="sc", name="sp")
                        nc.tensor.matmul(
                            sp[:, :n],
                            kTa[(b, h)][:, kt * 128:(kt + 1) * 128],
                            qTa[(b, h)][:, qg * SC_W + qlo:(qg + 1) * SC_W],
                            start=True, stop=True)
                        if kt >= qg * 4:
                            nc.vector.tensor_add(
                                sp[:, 0:128], sp[:, 0:128], mdg_sb[:, kt, :])
                        nc.scalar.activation(ptb[:, kt, qlo:SC_W], sp[:, :n], Exp)
                        if kt >= 1:
                            kl = kt - 1
                            ql2 = max(0, kl - qg * 4) * 128
                            nc.tensor.matmul(
                                po_ps[:, ql2:SC_W], vbfa[(b, h)][:, kl, :],
                                ptb[:, kl, ql2:SC_W],
                                start=(kl == 0), stop=False)
                    ql2 = max(0, kmax - qg * 4) * 128
                    nc.tensor.matmul(
                        po_ps[:, ql2:SC_W], vbfa[(b, h)][:, kmax, :],
                        ptb[:, kmax, ql2:SC_W], start=(kmax == 0), stop=True)
                    # in-place bf16 tree reduction over the kt axis -> ptb[:,0,:]
                    kk = K
                    while kk > 1:
                        m = kk // 2
                        nc.vector.tensor_add(ptb[:, 0:m, :], ptb[:, 0:m, :],
                                             ptb[:, kk - m:kk, :])
                        kk -= m
                    hstate[h] = (ptb, po_ps)
                pending[p] = (qg, b, hstate)

            # ============ deferred normalize + AllGather ============
            def normalize_chunk(p):
                qg, b, hstate = pending.pop(p)
                for h in range(HPC):
                    ptb, po_ps = hstate[h]
                    sb_ps = rotps.tile([128, SC_W], fp32, tag="rot", name="sb_ps")
                    nc.tensor.matmul(sb_ps[:], on2_sb[:], ptb[:, 0, :],
                                     start=True, stop=True)
                    rbc = smpool.tile([128, SC_W], fp32, tag="rbc", name="rbc")
                    nc.vector.reciprocal_approx_fast(rbc[:], sb_ps[:])
                    ob = smpool.tile([128, SC_W], bf16, tag="obuf", name="ob")
                    nc.vector.tensor_mul(ob[:], po_ps[:], rbc[:])
                    nc.sync.dma_start(ag_in_d[p][h * 128:(h + 1) * 128, :], ob[:])
                nc.gpsimd.collective_compute(
                    "AllGather", BYPASS, replica_groups=rg,
                    ins=[ag_in_d[p][:]], outs=[ag_out_d[p][:]])

            # ============ per-chunk O-projection (column-sharded) ============
            def oproj_chunk(p):
                slabs = []
                for e in range(DT):
                    agl = aglpool.tile([128, SC_W], bf16, tag="agl",
                                       name=f"agl{e}")
                    nc.sync.dma_start(agl[:],
                                      ag_out_d[p][e * 128:(e + 1) * 128, :])
                    slabs.append(agl)
                for dh in range(2):
                    op_ps = opps.tile([128, SC_W], fp32, tag="op", name="op_ps")
                    for e in range(DT):
                        nc.tensor.matmul(
                            op_ps[:],
                            wot_sb[:, e, dh * 128:(dh + 1) * 128],
                            slabs[e][:],
                            start=(e == 0), stop=(e == DT - 1))
                    obt = obpool.tile([128, SC_W], bf16, tag="ob", name="obt")
                    if dh == 0:
                        nc.scalar.copy(obt[:], op_ps[:])
                    else:
                        nc.vector.tensor_copy(obt[:], op_ps[:])
                    nc.sync.dma_start(
                        out_d[dh * 128:(dh + 1) * 128,
                              p * SC_W:(p + 1) * SC_W], obt[:])

            # ============ main fused loop ============
            for sc in range(NSC):
                b, qg = divmod(sc, 4)
                c0 = qg * SC_W               # column offset within batch
                if sc == 0:
                    xts = xts0
                else:
                    xts = []
                    for dt in range(DT):
                        xt = xpool.tile([128, SC_W], bf16, tag="xt",
                                        name=f"xt{dt}")
                        nc.sync.dma_start(
                            xt[:], xt_d[dt * 128:(dt + 1) * 128,
                                        sc * SC_W:(sc + 1) * SC_W])
                        xts.append(xt)

                def emit_rope_mm(t, h, til):
                    rp = rotps.tile([128, SC_W], fp32, tag="rot", name="rp")
                    nc.tensor.matmul(rp[:], rot_sb[:], til[:],
                                     start=True, stop=True)
                    dst = (qTa if t == "q" else kTa)[(b, h)]
                    t1 = tpool.tile([128, SC_W], bf16, tag="t1", name="t1")
                    nc.gpsimd.tensor_mul(t1[:], til[:], cos_sb[:, c0:c0 + SC_W])
                    hat = spool.tile([128, SC_W], bf16, tag="hat", name="hat")
                    nc.vector.tensor_mul(hat[:], rp[:], sin_sb[:, c0:c0 + SC_W])
                    nc.vector.tensor_add(dst[:, c0:c0 + SC_W], hat[:], t1[:])

                first_chain_done = False
                for h in range(HPC):
                    tils = {}
                    for t in ("q", "k"):
                        ps = qkvps.tile([128, SC_W], fp32, tag="qkv",
                                        name=f"ps_{t}")
                        for dt in range(DT):
                            nc.tensor.matmul(
                                ps[:],
                                w_sb[t][:, dt, h * HD:(h + 1) * HD],
                                xts[dt][:],
                                start=(dt == 0), stop=(dt == DT - 1))
                        if not first_chain_done:
                            first_chain_done = True
                            if sc == 0:
                                load_consts()
                            if sc >= 1:
                                # normalize previous chunk while this chunk's
                                # q-chain covers the PE
                                normalize_chunk(sc - 1)
                        til = spool.tile([128, SC_W], bf16, tag="til",
                                         name=f"til_{t}")
                        if t == "q":
                            nc.scalar.activation(til[:], ps[:], Copy,
                                                 scale=SCALE)
                        else:
                            nc.scalar.copy(til[:], ps[:])
                        tils[t] = til
                        if t == "k":
                            emit_rope_mm("q", h, tils["q"])
                    if h == 0:
                        # natural-layout V: stationary = x tile, both heads
                        # at once; no scalar input -> no PE stall risk
                        for vt in range(4):
                            psv = qkvps.tile([128, OC], fp32, tag="qkv",
                                             name="ps_v")
                            for dt in range(DT):
                                nc.tensor.matmul(
                                    psv[:],
                                    xts[dt][:, vt * 128:(vt + 1) * 128],
                                    w_sb["v"][:, dt, :],
                                    start=(dt == 0), stop=(dt == DT - 1))
                            for hh in range(HPC):
                                if hh == 0:
                                    nc.scalar.copy(
                                        vbfa[(b, hh)][:, qg * 4 + vt, :],
                                        psv[:, hh * HD:(hh + 1) * HD])
                                else:
                                    nc.vector.tensor_copy(
                                        vbfa[(b, hh)][:, qg * 4 + vt, :],
                                        psv[:, hh * HD:(hh + 1) * HD])
                    emit_rope_mm("k", h, tils["k"])

                attn_chunk(qg, b)
                if sc >= 3:
                    oproj_chunk(sc - 3)
            normalize_chunk(NSC - 1)
            oproj_chunk(NSC - 3)
            oproj_chunk(NSC - 2)
            oproj_chunk(NSC - 1)

    nc.compile()
    return nc


def _get_nc(S):
    if S not in _CACHE:
        _CACHE[S] = _build(S)
    return _CACHE[S]


def make_inputs(x, freqs_cis, mask, wq, wk, wv, wo):
    """Host-side sharding / layout prep. Returns in_maps for 8 cores."""
    S = x.shape[1]
    flat_xt = np.ascontiguousarray(np.asarray(x, np.float32).reshape(B * S, DIM).T)
    cos = np.asarray(freqs_cis[..., 0], np.float32)   # [S, HD/2]
    sin = np.asarray(freqs_cis[..., 1], np.float32)
    cos_t = np.ascontiguousarray(np.repeat(cos.T, 2, axis=0))  # [HD, S]
    sin_t = np.ascontiguousarray(np.repeat(sin.T, 2, axis=0))
    m = np.asarray(mask, np.float32)[0, 0]
    nqt = S // 128
    mask_diag = np.ascontiguousarray(
        np.stack([m[i * 128:(i + 1) * 128, i * 128:(i + 1) * 128].T
                  for i in range(nqt)]))
    import ml_dtypes
    bf = ml_dtypes.bfloat16
    flat_xt = flat_xt.astype(bf)
    cos_t = cos_t.astype(bf)
    sin_t = sin_t.astype(bf)
    P = np.zeros((128, 128), np.float32)
    for j in range(64):
        P[2 * j, 2 * j + 1] = -1.0
        P[2 * j + 1, 2 * j] = 1.0
    rotp = np.ascontiguousarray(P.T)

    wq = np.asarray(wq, np.float32)
    wk = np.asarray(wk, np.float32)
    wv = np.asarray(wv, np.float32)
    wo = np.asarray(wo, np.float32)
    in_maps = []
    for c in range(N_CORES):
        r = slice(c * OC, (c + 1) * OC)
        rd = slice(c * OCD, (c + 1) * OCD)
        in_maps.append({
            "xt": flat_xt,
            "wqt": np.ascontiguousarray(wq[r, :].T).astype(bf),
            "wkt": np.ascontiguousarray(wk[r, :].T).astype(bf),
            "wvt": np.ascontiguousarray(wv[r, :].T).astype(bf),
            "wotc": np.ascontiguousarray(wo[rd, :].T).astype(bf),
            "cos_t": cos_t,
            "sin_t": sin_t,
            "mask_diag": mask_diag.astype(bf),
            "rotp": rotp.astype(bf),
            "ones128": np.ones((128, 128), dtype=bf),
        })
    return in_maps


def assemble(results, S):
    """Column-concat per-core output shards into the full output."""
    full = np.empty((B * S, DIM), np.float32)
    for c in range(N_CORES):
        full[:, c * OCD:(c + 1) * OCD] = \
            np.asarray(results[c]["outT"], np.float32).T
    return full.reshape(B, S, DIM)


def kernel(x, start_pos, freqs_cis, mask, wq, wk, wv, wo):
    from concourse.bass_utils import run_bass_kernel_spmd
    S = x.shape[1]
    nc = _get_nc(S)
    in_maps = make_inputs(x, freqs_cis, mask, wq, wk, wv, wo)
    res = run_bass_kernel_spmd(nc, in_maps, core_ids=list(range(N_CORES)))
    return assemble(res.results, S)


# revision 11
# speedup vs baseline: 1.3628x; 1.0446x over previous
"""Trainium2 8-core tensor-parallel attention kernel (Bass/Tile), v3.

Sharding: heads tensor-parallel across 8 cores (2 heads/core) for
QKV + attention; output projection is column-sharded (each core owns
256 output channels) fed by per-chunk AllGathers of the pre-projection
attention outputs (2MB total exchanged vs 16.8MB for post-wo
ReduceScatter).

Single fused loop, interleaved to keep the PE p-state ramped:
  for sc in 0..7:                  # 512 flat seq rows; b = sc//4
    QKV q-chain for h=0
    normalize + AllGather of chunk sc-1   (softmax sums ready by now)
    rest of QKV (Q/K in [hd,seq] + RoPE, V in natural [seq,hd])
    attention q-group (scores/exp/AV; sums via in-place bf16 tree
    reduction on the contiguous probs buffer)
    O-projection of chunk sc-3            (AllGather long since done)

Self-contained: hardcodes B=2, S=2048, DIM=2048, NH=16, HD=128.
"""
import math

import numpy as np

B, S_FULL, DIM, NH = 2, 2048, 2048, 16
HD = 128
N_CORES = 8
HPC = NH // N_CORES          # heads per core (2)
OC = HPC * HD                # q/k/v channels per core (256)
OCD = DIM // N_CORES         # output channels per core (256)
DT = DIM // 128              # dim tiles (16)
SC_W = 512                   # schunk width (cols of flattened seq)

_CACHE = {}


def _build(S):
    """Build the 8-core SPMD Bass graph for sequence length S (B=2 fixed)."""
    import concourse.bass as bass
    import concourse.mybir as mybir
    import concourse.tile as tile
    from concourse import bacc

    fp32 = mybir.dt.float32
    bf16 = mybir.dt.bfloat16
    Exp = mybir.ActivationFunctionType.Exp
    Copy = mybir.ActivationFunctionType.Copy
    BYPASS = mybir.AluOpType.bypass

    FLAT = B * S                 # flattened rows (4096)
    NSC = FLAT // SC_W           # schunks / chunks (8)
    NQT = S // 128               # k-tiles per batch (16)
    SCALE = 1.0 / math.sqrt(HD)
    rg = [list(range(N_CORES))]

    nc = bacc.Bacc("TRN2", target_bir_lowering=False, debug=False,
                   num_devices=N_CORES)

    # ---- external parameters ----
    xt_d = nc.declare_dram_parameter("xt", [DIM, FLAT], bf16, isOutput=False)
    wqt_d = nc.declare_dram_parameter("wqt", [DIM, OC], bf16, isOutput=False)
    wkt_d = nc.declare_dram_parameter("wkt", [DIM, OC], bf16, isOutput=False)
    wvt_d = nc.declare_dram_parameter("wvt", [DIM, OC], bf16, isOutput=False)
    wot_d = nc.declare_dram_parameter("wotc", [DIM, OCD], bf16, isOutput=False)
    cos_d = nc.declare_dram_parameter("cos_t", [HD, S], bf16, isOutput=False)
    sin_d = nc.declare_dram_parameter("sin_t", [HD, S], bf16, isOutput=False)
    mdg_d = nc.declare_dram_parameter("mask_diag", [NQT, 128, 128], bf16, isOutput=False)
    rot_d = nc.declare_dram_parameter("rotp", [128, 128], bf16, isOutput=False)
    on2_d = nc.declare_dram_parameter("ones128", [128, 128], bf16, isOutput=False)
    out_d = nc.declare_dram_parameter("outT", [OCD, FLAT], bf16, isOutput=True)

    # ---- internal DRAM (collective staging) ----
    ag_in_d = [nc.dram_tensor(f"ag_in{p}", [OC, SC_W], bf16) for p in range(NSC)]
    ag_out_d = [nc.dram_tensor(f"ag_out{p}", [N_CORES * OC, SC_W], bf16,
                               addr_space="Shared") for p in range(NSC)]

    from contextlib import ExitStack
    with tile.TileContext(nc) as tc:
        with ExitStack() as _stk:
            cpool = _stk.enter_context(tc.tile_pool(name="consts", bufs=1))
            wpool = _stk.enter_context(tc.tile_pool(name="wqkv", bufs=1))
            xpool = _stk.enter_context(tc.tile_pool(name="xT", bufs=24))
            qkpool = _stk.enter_context(tc.tile_pool(name="qk_sb", bufs=1))
            vpool = _stk.enter_context(tc.tile_pool(name="vbf", bufs=1))
            spool = _stk.enter_context(tc.tile_pool(name="p1tmp", bufs=3))
            tpool = _stk.enter_context(tc.tile_pool(name="t1tmp", bufs=2))
            ptpool = _stk.enter_context(tc.tile_pool(name="probsT", bufs=1))
            smpool = _stk.enter_context(tc.tile_pool(name="small", bufs=2))
            aglpool = _stk.enter_context(tc.tile_pool(name="agl", bufs=20))
            obpool = _stk.enter_context(tc.tile_pool(name="outsb", bufs=4))
            qkvps = _stk.enter_context(tc.tile_pool(name="qkvps", bufs=2, space="PSUM"))
            rotps = _stk.enter_context(tc.tile_pool(name="rotps", bufs=1, space="PSUM"))
            scps = _stk.enter_context(tc.tile_pool(name="scps", bufs=2, space="PSUM"))
            pops = _stk.enter_context(tc.tile_pool(name="pops", bufs=2, space="PSUM"))
            opps = _stk.enter_context(tc.tile_pool(name="opps", bufs=1, space="PSUM"))

            # ---- weights + first x tiles interleaved (critical path) ----
            # per-dt weight tiles so the first chains start as soon as their
            # first slabs land, not after the whole 1MB tensor
            w_sb = {nm: [] for nm in ("q", "k", "v")}
            for nm, d in (("q", wqt_d), ("k", wkt_d), ("v", wvt_d)):
                for dt in range(DT):
                    w = wpool.tile([128, OC], bf16, tag=f"w{nm}{dt}",
                                   name=f"w{nm}{dt}")
                    w_sb[nm].append(w)
            xts0 = []
            for dt in range(DT):
                nc.sync.dma_start(w_sb["q"][dt][:],
                                  wqt_d[dt * 128:(dt + 1) * 128, :])
                xt = xpool.tile([128, SC_W], bf16, tag="xt", name=f"xt{dt}")
                nc.sync.dma_start(xt[:], xt_d[dt * 128:(dt + 1) * 128, 0:SC_W])
                xts0.append(xt)
            for dt in range(DT):
                nc.sync.dma_start(w_sb["k"][dt][:],
                                  wkt_d[dt * 128:(dt + 1) * 128, :])
            for dt in range(DT):
                nc.sync.dma_start(w_sb["v"][dt][:],
                                  wvt_d[dt * 128:(dt + 1) * 128, :])

            # persistent SBUF tensors
            qTa, kTa, vbfa = {}, {}, {}
            for bb in range(B):
                for h in range(HPC):
                    qTa[(bb, h)] = qkpool.tile([128, S], bf16, tag=f"qT{bb}{h}",
                                               name=f"qT{bb}{h}")
                    kTa[(bb, h)] = qkpool.tile([128, S], bf16, tag=f"kT{bb}{h}",
                                               name=f"kT{bb}{h}")
                    vbfa[(bb, h)] = vpool.tile([128, NQT, HD], bf16,
                                               tag=f"v{bb}{h}", name=f"v{bb}{h}")

            cos_sb = cpool.tile([HD, S], bf16)
            sin_sb = cpool.tile([HD, S], bf16)
            mdg_sb = cpool.tile([128, NQT, 128], bf16)
            rot_sb = cpool.tile([128, 128], bf16)
            on2_sb = cpool.tile([128, 128], bf16)
            wot_sb = cpool.tile([128, DT, OCD], bf16)

            def load_consts():
                nc.gpsimd.dma_start(cos_sb[:], cos_d[:])
                nc.gpsimd.dma_start(sin_sb[:], sin_d[:])
                nc.gpsimd.dma_start(mdg_sb[:], mdg_d[:].rearrange("t p k -> p t k"))
                nc.gpsimd.dma_start(rot_sb[:], rot_d[:])
                nc.gpsimd.dma_start(on2_sb[:], on2_d[:])
                for dt in range(DT):
                    nc.sync.dma_start(wot_sb[:, dt, :],
                                      wot_d[dt * 128:(dt + 1) * 128, :])

            # state carried between loop iterations for deferred normalize
            pending = {}   # chunk p -> (qg, b, {h: (pt_buf, po_ps)})

            # ============ per-chunk attention (h sections only) ============
            def attn_chunk(qg, b):
                kmax = qg * 4 + 3
                K = kmax + 1
                p = b * 4 + qg
                hstate = {}
                for h in range(HPC):
                    po_ps = pops.tile([128, SC_W], fp32, tag="po", name=f"po{h}")
                    ptb = ptpool.tile([128, NQT, SC_W], bf16, tag=f"pTb{h}",
                                      name=f"pTb{h}")
                    # zero the above-diagonal cols of the partial tiles so the
                    # tree reduction sees exact zeros there
                    for kt in range(qg * 4 + 1, kmax + 1):
                        qlo = (kt - qg * 4) * 128
                        nc.gpsimd.memset(ptb[:, kt, 0:qlo], 0)
                    for kt in range(K):
                        qlo = max(0, kt - qg * 4) * 128
                        n = SC_W - qlo
                        sp = scps.tile([128, SC_W], fp32, tag="sc", name="sp")
                        nc.tensor.matmul(
                            sp[:, :n],
                            kTa[(b, h)][:, kt * 128:(kt + 1) * 128],
                            qTa[(b, h)][:, qg * SC_W + qlo:(qg + 1) * SC_W],
                            start=True, stop=True)
                        if kt >= qg * 4:
                            nc.vector.tensor_add(
                                sp[:, 0:128], sp[:, 0:128], mdg_sb[:, kt, :])
                        nc.scalar.activation(ptb[:, kt, qlo:SC_W], sp[:, :n], Exp)
                        if kt >= 1:
                            kl = kt - 1
                            ql2 = max(0, kl - qg * 4) * 128
                            nc.tensor.matmul(
                                po_ps[:, ql2:SC_W], vbfa[(b, h)][:, kl, :],
                                ptb[:, kl, ql2:SC_W],
                                start=(kl == 0), stop=False)
                    ql2 = max(0, kmax - qg * 4) * 128
                    nc.tensor.matmul(
                        po_ps[:, ql2:SC_W], vbfa[(b, h)][:, kmax, :],
                        ptb[:, kmax, ql2:SC_W], start=(kmax == 0), stop=True)
                    # in-place bf16 tree reduction over the kt axis -> ptb[:,0,:]
                    kk = K
                    while kk > 1:
                        m = kk // 2
                        nc.vector.tensor_add(ptb[:, 0:m, :], ptb[:, 0:m, :],
                                             ptb[:, kk - m:kk, :])
                        kk -= m
                    hstate[h] = (ptb, po_ps)
                pending[p] = (qg, b, hstate)

            # ============ deferred normalize + AllGather ============
            def normalize_chunk(p):
                qg, b, hstate = pending.pop(p)
                for h in range(HPC):
                    ptb, po_ps = hstate[h]
                    sb_ps = rotps.tile([128, SC_W], fp32, tag="rot", name="sb_ps")
                    nc.tensor.matmul(sb_ps[:], on2_sb[:], ptb[:, 0, :],
                                     start=True, stop=True)
                    rbc = smpool.tile([128, SC_W], fp32, tag="rbc", name="rbc")
                    nc.vector.reciprocal_approx_fast(rbc[:], sb_ps[:])
                    ob = smpool.tile([128, SC_W], bf16, tag="obuf", name="ob")
                    nc.vector.tensor_mul(ob[:], po_ps[:], rbc[:])
                    nc.sync.dma_start(ag_in_d[p][h * 128:(h + 1) * 128, :], ob[:])
                nc.gpsimd.collective_compute(
                    "AllGather", BYPASS, replica_groups=rg,
                    ins=[ag_in_d[p][:]], outs=[ag_out_d[p][:]])

            # ============ per-chunk O-projection (column-sharded) ============
            def oproj_chunk(p):
                slabs = []
                for e in range(DT):
                    agl = aglpool.tile([128, SC_W], bf16, tag="agl",
                                       name=f"agl{e}")
                    nc.sync.dma_start(agl[:],
                                      ag_out_d[p][e * 128:(e + 1) * 128, :])
                    slabs.append(agl)
                for dh in range(2):
                    # dh=1 borrows the rot pool's bank so the two half-chains
                    # don't serialize on a single PSUM drain
                    pool = opps if dh == 0 else rotps
                    op_ps = pool.tile([128, SC_W], fp32,
                                      tag="op" if dh == 0 else "rot",
                                      name="op_ps")
                    for e in range(DT):
                        nc.tensor.matmul(
                            op_ps[:],
                            wot_sb[:, e, dh * 128:(dh + 1) * 128],
                            slabs[e][:],
                            start=(e == 0), stop=(e == DT - 1))
                    obt = obpool.tile([128, SC_W], bf16, tag="ob", name="obt")
                    if dh == 0:
                        nc.scalar.copy(obt[:], op_ps[:])
                    else:
                        nc.vector.tensor_copy(obt[:], op_ps[:])
                    nc.sync.dma_start(
                        out_d[dh * 128:(dh + 1) * 128,
                              p * SC_W:(p + 1) * SC_W], obt[:])

            # ============ main fused loop ============
            for sc in range(NSC):
                b, qg = divmod(sc, 4)
                c0 = qg * SC_W               # column offset within batch
                if sc == 0:
                    xts = xts0
                else:
                    xts = []
                    for dt in range(DT):
                        xt = xpool.tile([128, SC_W], bf16, tag="xt",
                                        name=f"xt{dt}")
                        nc.sync.dma_start(
                            xt[:], xt_d[dt * 128:(dt + 1) * 128,
                                        sc * SC_W:(sc + 1) * SC_W])
                        xts.append(xt)

                def emit_rope_mm(t, h, til):
                    rp = rotps.tile([128, SC_W], fp32, tag="rot", name="rp")
                    nc.tensor.matmul(rp[:], rot_sb[:], til[:],
                                     start=True, stop=True)
                    dst = (qTa if t == "q" else kTa)[(b, h)]
                    t1 = tpool.tile([128, SC_W], bf16, tag="t1", name="t1")
                    nc.gpsimd.tensor_mul(t1[:], til[:], cos_sb[:, c0:c0 + SC_W])
                    hat = spool.tile([128, SC_W], bf16, tag="hat", name="hat")
                    nc.vector.tensor_mul(hat[:], rp[:], sin_sb[:, c0:c0 + SC_W])
                    nc.vector.tensor_add(dst[:, c0:c0 + SC_W], hat[:], t1[:])

                first_chain_done = False
                for h in range(HPC):
                    tils = {}
                    for t in ("q", "k"):
                        ps = qkvps.tile([128, SC_W], fp32, tag="qkv",
                                        name=f"ps_{t}")
                        for dt in range(DT):
                            nc.tensor.matmul(
                                ps[:],
                                w_sb[t][dt][:, h * HD:(h + 1) * HD],
                                xts[dt][:],
                                start=(dt == 0), stop=(dt == DT - 1))
                        if not first_chain_done:
                            first_chain_done = True
                            if sc == 0:
                                load_consts()
                            if sc >= 1:
                                # normalize previous chunk while this chunk's
                                # q-chain covers the PE
                                normalize_chunk(sc - 1)
                        til = spool.tile([128, SC_W], bf16, tag="til",
                                         name=f"til_{t}")
                        if t == "q":
                            nc.scalar.activation(til[:], ps[:], Copy,
                                                 scale=SCALE)
                        else:
                            nc.scalar.copy(til[:], ps[:])
                        tils[t] = til
                        if t == "k":
                            emit_rope_mm("q", h, tils["q"])
                    if h == 0:
                        # natural-layout V: stationary = x tile, both heads
                        # at once; no scalar input -> no PE stall risk
                        for vt in range(4):
                            psv = qkvps.tile([128, OC], fp32, tag="qkv",
                                             name="ps_v")
                            for dt in range(DT):
                                nc.tensor.matmul(
                                    psv[:],
                                    xts[dt][:, vt * 128:(vt + 1) * 128],
                                    w_sb["v"][dt][:],
                                    start=(dt == 0), stop=(dt == DT - 1))
                            for hh in range(HPC):
                                if hh == 0:
                                    nc.scalar.copy(
                                        vbfa[(b, hh)][:, qg * 4 + vt, :],
                                        psv[:, hh * HD:(hh + 1) * HD])
                                else:
                                    nc.vector.tensor_copy(
                                        vbfa[(b, hh)][:, qg * 4 + vt, :],
                                        psv[:, hh * HD:(hh + 1) * HD])
                    emit_rope_mm("k", h, tils["k"])

                attn_chunk(qg, b)
                if sc >= 4:
                    oproj_chunk(sc - 4)
            normalize_chunk(NSC - 1)
            oproj_chunk(NSC - 4)
            oproj_chunk(NSC - 3)
            oproj_chunk(NSC - 2)
            oproj_chunk(NSC - 1)

    nc.compile()
    return nc


def _get_nc(S):
    if S not in _CACHE:
        _CACHE[S] = _build(S)
    return _CACHE[S]


def make_inputs(x, freqs_cis, mask, wq, wk, wv, wo):
    """Host-side sharding / layout prep. Returns in_maps for 8 cores."""
    S = x.shape[1]
    flat_xt = np.ascontiguousarray(np.asarray(x, np.float32).reshape(B * S, DIM).T)
    cos = np.asarray(freqs_cis[..., 0], np.float32)   # [S, HD/2]
    sin = np.asarray(freqs_cis[..., 1], np.float32)
    cos_t = np.ascontiguousarray(np.repeat(cos.T, 2, axis=0))  # [HD, S]
    sin_t = np.ascontiguousarray(np.repeat(sin.T, 2, axis=0))
    m = np.asarray(mask, np.float32)[0, 0]
    nqt = S // 128
    mask_diag = np.ascontiguousarray(
        np.stack([m[i * 128:(i + 1) * 128, i * 128:(i + 1) * 128].T
                  for i in range(nqt)]))
    import ml_dtypes
    bf = ml_dtypes.bfloat16
    flat_xt = flat_xt.astype(bf)
    cos_t = cos_t.astype(bf)
    sin_t = sin_t.astype(bf)
    P = np.zeros((128, 128), np.float32)
    for j in range(64):
        P[2 * j, 2 * j + 1] = -1.0
        P[2 * j + 1, 2 * j] = 1.0
    rotp = np.ascontiguousarray(P.T)

    wq = np.asarray(wq, np.float32)
    wk = np.asarray(wk, np.float32)
    wv = np.asarray(wv, np.float32)
    wo = np.asarray(wo, np.float32)
    in_maps = []
    for c in range(N_CORES):
        r = slice(c * OC, (c + 1) * OC)
        rd = slice(c * OCD, (c + 1) * OCD)
        in_maps.append({
            "xt": flat_xt,
            "wqt": np.ascontiguousarray(wq[r, :].T).astype(bf),
            "wkt": np.ascontiguousarray(wk[r, :].T).astype(bf),
            "wvt": np.ascontiguousarray(wv[r, :].T).astype(bf),
            "wotc": np.ascontiguousarray(wo[rd, :].T).astype(bf),
            "cos_t": cos_t,
            "sin_t": sin_t,
            "mask_diag": mask_diag.astype(bf),
            "rotp": rotp.astype(bf),
            "ones128": np.ones((128, 128), dtype=bf),
        })
    return in_maps


def assemble(results, S):
    """Column-concat per-core output shards into the full output."""
    full = np.empty((B * S, DIM), np.float32)
    for c in range(N_CORES):
        full[:, c * OCD:(c + 1) * OCD] = \
            np.asarray(results[c]["outT"], np.float32).T
    return full.reshape(B, S, DIM)


def kernel(x, start_pos, freqs_cis, mask, wq, wk, wv, wo):
    from concourse.bass_utils import run_bass_kernel_spmd
    S = x.shape[1]
    nc = _get_nc(S)
    in_maps = make_inputs(x, freqs_cis, mask, wq, wk, wv, wo)
    res = run_bass_kernel_spmd(nc, in_maps, core_ids=list(range(N_CORES)))
    return assemble(res.results, S)
